# revision 2
# baseline (speedup 1.0000x reference)
"""MoE routing + expert FFN kernel for 8 Trainium2 NeuronCores — v2.

Sharding: data-parallel routing (core g owns token group g) + expert-parallel
FFN (core e owns expert e); dispatch/combine are on-device AllToAlls.

v2 restructure vs v1 (362.7us cost model):
  - Host pre-packs every streamed tensor into the exact SBUF layout so each
    load is ONE contiguous whole-partition-line DMA (descriptor-gen and
    DGE-queue time dominated the v1 head).
  - Dispatch matmul emits the TRANSPOSED layout xdispT[e, h, c] directly
    (lhsT = token tile, moving = one-hot slot mask); the post-A2A xbar
    transpose disappears. A2A#1 is split into cap-halves so M1 starts
    after the first half lands.
  - M2/A2A#2/combine split into four h-quarters: each quarter's collective,
    gather, scale and store hide under the next quarter's matmuls.
  - yy/ycomb in bf16; batched combine gather (one indirect DMA per quarter).
  - Queues: SP carries token/w1/w2/xt streams; ACT carries consts, xdispT,
    yy and out writes; gpsimd carries collectives + gathers.
"""

import sys

sys.path.insert(0, "/opt/trn_rl_repo")

import numpy as np
import ml_dtypes

G, T, H, E, DFF, CAP = 8, 1024, 1024, 8, 4096, 128
NCORES = 8
P = 128
CH = CAP // 2  # cap half per A2A#1 piece
HQ = H // 4    # h quarter per A2A#2 piece
NT = T // P    # 8 token tiles per group

_STATE = {}
DISPATCH_MODE = "pe"  # "scatter_batched" | "scatter_tile" | "pe"
BATCHED_GATHER = False


def _build_nc(fake_collectives=False, stages=None, dispatch_mode=None,
              batched_gather=None):
    if dispatch_mode is None:
        dispatch_mode = DISPATCH_MODE
    if batched_gather is None:
        batched_gather = BATCHED_GATHER
    from concourse import bacc
    import concourse.bass as bass
    import concourse.mybir as mybir
    import concourse.tile as tile

    f32 = mybir.dt.float32
    bf16 = mybir.dt.bfloat16
    i32 = mybir.dt.int32
    X = mybir.AxisListType.X
    AF = mybir.ActivationFunctionType
    OP = mybir.AluOpType

    nc = bacc.Bacc("TRN2", target_bir_lowering=False, debug=False,
                   num_devices=NCORES)

    # host-prepacked inputs (already in SBUF layout, partition dim first)
    tok_t1 = nc.dram_tensor("tok_t1", [P, 8 * 512], f32, kind="ExternalInput")
    tok_t2 = nc.dram_tensor("tok_t2", [P, 8 * 512], f32, kind="ExternalInput")
    tok_bp = nc.dram_tensor("tok_bp", [P, NT * H], bf16, kind="ExternalInput")
    wrp = nc.dram_tensor("wrp", [P, 8 * E], f32, kind="ExternalInput")
    w1p_d = nc.dram_tensor("w1p", [P, 8 * DFF], bf16, kind="ExternalInput")
    w2p_d = nc.dram_tensor("w2p", [P, 8 * DFF], bf16, kind="ExternalInput")
    consts = nc.dram_tensor("consts", [P, 2 * P + 8 * E + 512], f32,
                            kind="ExternalInput")
    out = nc.dram_tensor("out", [T, H], f32, kind="ExternalOutput")

    # A2A#1 carried as [E*CH, H] cap-halves (rows e*CH+c, +1 dump row);
    # A2A#2 as [T(+1), HQ] h-quarters.
    if dispatch_mode == "pe":
        xdispT = [[nc.dram_tensor(f"xdispT{i}_{j}", [E, H // 2, CH], bf16)
                   for j in range(2)] for i in range(2)]
        xrecvT = [[nc.dram_tensor(f"xrecvT{i}_{j}", [G, H // 2, CH], bf16)
                   for j in range(2)] for i in range(2)]
    else:
        xdisp = [nc.dram_tensor(f"xdisp{i}", [E * CH + 1, H], bf16)
                 for i in range(2)]
        xrecv = [nc.dram_tensor(f"xrecv{i}", [G * CH, H], bf16)
                 for i in range(2)]
    yy = [nc.dram_tensor(f"yy{i}", [T, HQ], bf16) for i in range(4)]
    ycomb = [nc.dram_tensor(f"ycomb{i}", [T + 1, HQ], bf16) for i in range(4)]

    RG = [list(range(NCORES))]
    ALL = {"router", "cumsum", "dispatch", "m1", "m2", "combine"}
    stg = ALL if stages is None else set(stages)

    def _n(stage, n):
        return n if stage in stg else 0

    def a2a(in_t, out_t):
        if fake_collectives:
            nc.gpsimd.dma_start(out=out_t, in_=in_t)
        else:
            nc.gpsimd.collective_compute(
                "AllToAll", mybir.AluOpType.bypass, replica_groups=RG,
                ins=[in_t.opt()], outs=[out_t.opt()])

    with tile.TileContext(nc) as tc:
        with (
            tc.tile_pool(name="const", bufs=1) as constp,
            tc.tile_pool(name="big", bufs=1) as big,
            tc.tile_pool(name="rt", bufs=2) as rtp,
            tc.tile_pool(name="w1s_p", bufs=3) as w1pool,
            tc.tile_pool(name="w2s_p", bufs=2) as w2pool,
            tc.tile_pool(name="io", bufs=4) as iop,
            tc.tile_pool(name="cbp", bufs=2) as cbp,
            tc.tile_pool(name="ps1", bufs=2, space="PSUM") as ps1,
            tc.tile_pool(name="ps2", bufs=1, space="PSUM") as ps2,
        ):
            # ---- small consts first (they gate router/meta), then tokens on
            # both queues; tokb (needed only at dispatch) trails on SP
            tokT_sb = big.tile([P, 8 * T], f32)
            tokb_sb = big.tile([P, NT * H], bf16)
            wr_sb = constp.tile([P, 8 * E], f32)
            nc.scalar.dma_start(wr_sb[:], wrp[:, :])
            call = constp.tile([P, 2 * P + 8 * E + 512], f32)
            nc.scalar.dma_start(call[:], consts[:, :])
            nc.sync.dma_start(
                tokT_sb[:].rearrange("p (k t) -> p k t", k=8)[:, :, :512],
                tok_t1[:, :].rearrange("p (k t) -> p k t", k=8))
            nc.sync.dma_start(
                tokT_sb[:].rearrange("p (k t) -> p k t", k=8)[:, :, 512:],
                tok_t2[:, :].rearrange("p (k t) -> p k t", k=8))
            nc.sync.dma_start(tokb_sb[:], tok_bp[:, :])

            ones_sb = constp.tile([P, P], bf16)
            nc.vector.tensor_copy(ones_sb[:], call[:, 0:P])
            utri_sb = constp.tile([P, P], bf16)
            nc.vector.tensor_copy(utri_sb[:], call[:, P:2 * P])
            iota_sb = call[:, 2 * P:2 * P + 8 * E]
            siota_sb = call[:, 2 * P + 8 * E:]
            zrow = constp.tile([1, HQ], bf16)
            nc.vector.memset(zrow[:], 0.0)
            for i in range(4):
                nc.scalar.dma_start(ycomb[i][T:T + 1, :], zrow[:, :])

            # ---- router: per-token-tile k-chains (psr bufs=2 pingpong)
            lg = rtp.tile([P, NT * E], f32)
            for m in range(_n("router", NT)):
                lg_ps = ps1.tile([P, E], f32, name="lg_ps", tag="hps")
                for k in range(8):
                    nc.tensor.matmul(
                        lg_ps[:],
                        lhsT=tokT_sb[:, k * T + m * P: k * T + (m + 1) * P],
                        rhs=wr_sb[:, k * E:(k + 1) * E],
                        start=(k == 0), stop=(k == 7))
                nc.vector.tensor_copy(lg[:, m * E:(m + 1) * E], lg_ps[:])

            maskb = big.tile([P, NT * E], bf16)
            maskf_all = big.tile([P, NT * E], f32)
            gate_all = big.tile([P, NT], f32)
            idx_all = big.tile([P, NT], f32)
            addr_i = big.tile([P, NT], i32)
            scale_all = big.tile([P, NT], f32)
            addr_f = big.tile([P, NT], f32)
            addr_hf = [big.tile([P, NT], f32, name=f"addr_hf{i}")
                       for i in range(2)]
            addr_hi = [big.tile([P, NT], i32, name=f"addr_hi{i}")
                       for i in range(2)]

            if "router" in stg:
                mx = rtp.tile([P, NT], f32)
                nc.vector.tensor_reduce(
                    mx[:], lg[:].rearrange("p (m e) -> p m e", e=E), axis=X,
                    op=OP.max)
                mxb = mx[:].rearrange("p m -> p m ()").broadcast_to([P, NT, E])
                lg3 = lg[:].rearrange("p (m e) -> p m e", e=E)
                # mask = (logit >= rowmax), as f32 and bf16
                nc.vector.tensor_tensor(
                    maskf_all[:].rearrange("p (m e) -> p m e", e=E),
                    lg3, mxb, op=OP.is_ge)
                nc.vector.tensor_copy(maskb[:], maskf_all[:])
                # exp(logit - rowmax), summed over e -> 1/gate
                exm = rtp.tile([P, NT * E], f32)
                nc.vector.tensor_tensor(
                    exm[:].rearrange("p (m e) -> p m e", e=E),
                    lg3, mxb, op=OP.subtract)
                ex = rtp.tile([P, NT * E], f32)
                nc.scalar.activation(ex[:], exm[:], AF.Exp)
                esum = rtp.tile([P, NT], f32)
                nc.vector.reduce_sum(
                    esum[:], ex[:].rearrange("p (m e) -> p m e", e=E), axis=X)
                nc.vector.reciprocal(gate_all[:], esum[:])
                # expert index = sum(mask * iota)
                iw = rtp.tile([P, NT * E], f32)
                nc.vector.tensor_tensor(iw[:], maskf_all[:], iota_sb,
                                        op=OP.mult)
                nc.vector.reduce_sum(
                    idx_all[:], iw[:].rearrange("p (m e) -> p m e", e=E),
                    axis=X)

            # ---- capacity positions: utri within tile + running col sums
            cum_all = big.tile([P, NT * E], f32)
            for m in range(_n("cumsum", NT)):
                cum_ps = ps1.tile([P, E], f32, name="cum_ps", tag="hps")
                for k in range(m + 1):
                    nc.tensor.matmul(
                        cum_ps[:],
                        lhsT=(utri_sb[:] if k == m else ones_sb[:]),
                        rhs=maskb[:, k * E:(k + 1) * E],
                        start=(k == 0), stop=(k == m))
                nc.vector.tensor_copy(cum_all[:, m * E:(m + 1) * E], cum_ps[:])
            if "cumsum" in stg:
                mcum = rtp.tile([P, NT * E], f32)
                nc.vector.tensor_tensor(mcum[:], maskf_all[:], cum_all[:],
                                        op=OP.mult)
                pos = rtp.tile([P, NT], f32)
                nc.vector.reduce_sum(
                    pos[:], mcum[:].rearrange("p (m e) -> p m e", e=E), axis=X)
                nc.vector.tensor_scalar_sub(pos[:], pos[:], 1.0)
                kept = rtp.tile([P, NT], f32)
                nc.vector.tensor_scalar(kept[:], pos[:], float(CAP), None,
                                        op0=OP.is_lt)
                # per-cap-half addresses FIRST (they gate dmask/dispatch):
                # addr_h = (idx*CH + pos - half*CH) if half owns pos else 512
                ish = rtp.tile([P, NT], f32, name="ish")
                base = [rtp.tile([P, NT], f32, name=f"base{i}")
                        for i in range(2)]
                for half in range(2):
                    if half == 0:
                        nc.vector.tensor_scalar(ish[:], pos[:], float(CH),
                                                None, op0=OP.is_lt)
                    else:
                        nc.vector.tensor_scalar(ish[:], pos[:], float(CH),
                                                None, op0=OP.is_ge)
                        nc.vector.tensor_tensor(ish[:], ish[:], kept[:],
                                                op=OP.mult)
                    nc.vector.tensor_scalar_mul(base[half][:], idx_all[:],
                                                float(CH))
                    nc.vector.tensor_tensor(base[half][:], base[half][:],
                                            pos[:], op=OP.add)
                    nc.vector.tensor_scalar_sub(
                        base[half][:], base[half][:], float(half * CH + 512))
                    nc.vector.tensor_tensor(base[half][:], base[half][:],
                                            ish[:], op=OP.mult)
                    nc.vector.tensor_scalar(addr_hf[half][:], base[half][:],
                                            512.0, 0.0, op0=OP.add,
                                            op1=OP.max)
                # combine-side addresses + gate scale (needed much later)
                drop = rtp.tile([P, NT], f32)
                nc.vector.tensor_scalar(drop[:], pos[:], float(CAP), None,
                                        op0=OP.is_ge)
                nc.vector.tensor_scalar_mul(addr_f[:], idx_all[:], float(CAP))
                nc.vector.tensor_tensor(addr_f[:], addr_f[:], pos[:],
                                        op=OP.add)
                nc.vector.tensor_tensor(addr_f[:], addr_f[:], kept[:],
                                        op=OP.mult)
                nc.vector.tensor_scalar_mul(drop[:], drop[:], float(T))
                nc.vector.tensor_tensor(addr_f[:], addr_f[:], drop[:],
                                        op=OP.add)
                nc.vector.tensor_scalar_max(addr_f[:], addr_f[:], 0.0)
                nc.vector.tensor_scalar_min(addr_f[:], addr_f[:], float(T))
                nc.vector.tensor_copy(addr_i[:], addr_f[:])
                nc.vector.tensor_tensor(scale_all[:], gate_all[:], kept[:],
                                        op=OP.mult)
                for half in range(2):
                    nc.vector.tensor_copy(addr_hi[half][:], addr_hf[half][:])

            # ---- dispatch per cap-half: either an indirect row-scatter
            # (token rows -> slot rows e*CH+c, dropped -> dump row 512), or a
            # PE one-hot matmul emitting the transposed layout directly
            if dispatch_mode == "pe":
                dmask = big.tile([P, NT * T], bf16)

                def build_dmask(half):
                    for m in range(_n("dispatch", NT)):
                        nc.vector.tensor_scalar(
                            dmask[:, m * T + half * 512:
                                  m * T + (half + 1) * 512],
                            siota_sb, addr_hf[half][:, m:m + 1],
                            None, op0=OP.is_equal)
                build_dmask(0)
            for half in range(_n("dispatch", 2)):
                if dispatch_mode == "pe":
                    for hb in range(8):
                        dps = ps1.tile([P, 512], f32, name="dps", tag="hps")
                        for tb in range(NT):
                            nc.tensor.matmul(
                                dps[:],
                                lhsT=tokb_sb[:, tb * H + hb * P:
                                             tb * H + (hb + 1) * P],
                                rhs=dmask[:, tb * T + half * 512:
                                          tb * T + (half + 1) * 512],
                                start=(tb == 0), stop=(tb == NT - 1))
                        xo = iop.tile([P, 512], bf16, name="xo", tag="xo")
                        nc.scalar.activation(xo[:], dps[:], AF.Copy)
                        nc.scalar.dma_start(
                            xdispT[half][hb // 4]
                            [:, (hb % 4) * P:(hb % 4 + 1) * P, :]
                            .transpose([1, 0, 2]),
                            xo[:].rearrange("p (e c) -> p e c", c=CH))
                        if hb == 0 and half == 0:
                            build_dmask(1)
                        if hb == 3:
                            a2a(xdispT[half][0][:, :, :],
                                xrecvT[half][0][:, :, :])
                    a2a(xdispT[half][1][:, :, :], xrecvT[half][1][:, :, :])
                elif dispatch_mode == "scatter_batched":
                    nc.gpsimd.indirect_dma_start(
                        out=xdisp[half][:, :],
                        out_offset=bass.IndirectOffsetOnAxis(
                            ap=addr_hi[half][:, :], axis=0),
                        in_=tokb_sb[:].rearrange("p (m h) -> p m h", h=H),
                        in_offset=None)
                    a2a(xdisp[half][0:E * CH, :], xrecv[half][:, :])
                else:
                    for m in range(NT):
                        nc.gpsimd.indirect_dma_start(
                            out=xdisp[half][:, :],
                            out_offset=bass.IndirectOffsetOnAxis(
                                ap=addr_hi[half][:, m:m + 1], axis=0),
                            in_=tokb_sb[:, m * H:(m + 1) * H],
                            in_offset=None)
                    a2a(xdisp[half][0:E * CH, :], xrecv[half][:, :])

            # ---- M1: hT[dff, slot] = relu(w1.T @ x) per cap-half
            # slot columns within ht_sb: (g, c) with c global (0..127)
            # w1s loads software-pipelined 2 deep ahead of the compute
            ht_sb = big.tile([P, 32 * T], bf16)
            w1s_tiles = {}

            def load_w1(mb):
                t = w1pool.tile([P, 8 * 512], bf16, name="w1s")
                nc.sync.dma_start(t[:], w1p_d[:, mb * 4096:(mb + 1) * 4096])
                return t

            nw1 = _n("m1", 2) * 8
            xt_sbs = [big.tile([P, 8 * 512], bf16, name=f"xt_sb{i}")
                      for i in range(2)]

            def stage_xt(half):
                for k in range(8):
                    if dispatch_mode == "pe":
                        nc.sync.dma_start(
                            xt_sbs[half][:, k * 512:(k + 1) * 512]
                            .rearrange("p (g c) -> p g c", c=CH),
                            xrecvT[half][k // 4]
                            [:, (k % 4) * P:(k % 4 + 1) * P, :]
                            .transpose([1, 0, 2]))
                    else:
                        nc.sync.dma_start_transpose(
                            xt_sbs[half][:, k * 512:(k + 1) * 512],
                            xrecv[half][:, k * P:(k + 1) * P])

            if _n("m1", 2):
                stage_xt(0)
            for half in range(_n("m1", 2)):
                xt_sb = xt_sbs[half]
                for mb in range(8):
                    if half == 0 and mb == 4:
                        stage_xt(1)
                    j = half * 8 + mb
                    if j == 0:
                        for jj in range(min(2, nw1)):
                            w1s_tiles[jj] = load_w1(jj % 8)
                    w1s = w1s_tiles.pop(j)
                    if j + 2 < nw1:
                        w1s_tiles[j + 2] = load_w1((j + 2) % 8)
                    for m4 in range(4):
                        mm = mb * 4 + m4
                        hps = ps1.tile([P, 512], f32, name="hps", tag="hps")
                        for k in range(8):
                            nc.tensor.matmul(
                                hps[:],
                                lhsT=w1s[:, k * 512 + m4 * P:
                                         k * 512 + (m4 + 1) * P],
                                rhs=xt_sb[:, k * 512:(k + 1) * 512],
                                start=(k == 0), stop=(k == 7))
                        nc.scalar.activation(
                            ht_sb[:, mm * T:(mm + 1) * T]
                            .rearrange("p (g c) -> p g c", c=CAP)
                            [:, :, half * CH:(half + 1) * CH],
                            hps[:], AF.Relu)

            # ---- M2: yy[slot, h] = hT.T @ w2 per h-quarter; slot tile = group
            for hq in range(_n("m2", 4)):
                for tmb in range(2):
                    pss = [ps2.tile([P, 256], f32, name=f"pss{i}",
                                    tag=f"pss{i}", bufs=2 if i < 2 else 1)
                           for i in range(4)]
                    for kb in range(4):
                        w2s = w2pool.tile([P, 8 * 256], bf16)
                        with tc.tile_wait_until(
                                0.040, enable=(hq == 0 and tmb == 0
                                               and kb < 2)):
                            nc.sync.dma_start(
                                w2s[:], w2p_d[:, (hq * 4 + kb) * 2048:
                                              (hq * 4 + kb + 1) * 2048])
                        for t4 in range(4):
                            tm = tmb * 4 + t4
                            for k in range(8):
                                kk = kb * 8 + k
                                nc.tensor.matmul(
                                    pss[t4][:],
                                    lhsT=ht_sb[:, kk * T + tm * P:
                                               kk * T + (tm + 1) * P],
                                    rhs=w2s[:, k * 256:(k + 1) * 256],
                                    start=(kk == 0), stop=(kk == 31))
                    for t4 in range(4):
                        tm = tmb * 4 + t4
                        yo = iop.tile([P, 256], bf16, name="yo", tag="yo")
                        if t4 % 2 == 0:
                            nc.vector.tensor_copy(yo[:], pss[t4][:])
                        else:
                            nc.scalar.activation(yo[:], pss[t4][:], AF.Copy)
                        nc.scalar.dma_start(yy[hq][tm * P:(tm + 1) * P, :],
                                            yo[:])
                # ---- A2A#2 + combine gather for this h-quarter
                if "m2" in stg:
                    a2a(yy[hq][:, :], ycomb[hq][0:T, :])
                if "combine" in stg:
                    # gather y rows by token address; pipeline gathers with
                    # the scale+store chunks, writes on alternating queues
                    cb = cbp.tile([P, NT * HQ], bf16, name="cb", tag="cb",
                                  bufs=2)
                    if batched_gather:
                        nc.gpsimd.indirect_dma_start(
                            out=cb[:].rearrange("p (m c) -> p m c", c=HQ),
                            out_offset=None,
                            in_=ycomb[hq][:, :],
                            in_offset=bass.IndirectOffsetOnAxis(
                                ap=addr_i[:, :], axis=0))
                    for q in range(2):
                        m0 = q * 4
                        if not batched_gather:
                            for m in range(m0, m0 + 4):
                                nc.gpsimd.indirect_dma_start(
                                    out=cb[:, m * HQ:(m + 1) * HQ],
                                    out_offset=None,
                                    in_=ycomb[hq][:, :],
                                    in_offset=bass.IndirectOffsetOnAxis(
                                        ap=addr_i[:, m:m + 1], axis=0))
                        cf = cbp.tile([P, 4 * HQ], f32, name="cf", tag="cf",
                                      bufs=2)
                        nc.vector.tensor_tensor(
                            cf[:].rearrange("p (m c) -> p m c", c=HQ),
                            cb[:, m0 * HQ:(m0 + 4) * HQ]
                            .rearrange("p (m c) -> p m c", c=HQ),
                            scale_all[:, m0:m0 + 4]
                            .rearrange("p m -> p m ()")
                            .broadcast_to([P, 4, HQ]),
                            op=OP.mult)
                        (nc.sync if q == 0 else nc.scalar).dma_start(
                            out[m0 * P:(m0 + 4) * P,
                                hq * HQ:(hq + 1) * HQ].rearrange(
                                "(m p) c -> p m c", p=P),
                            cf[:].rearrange("p (m c) -> p m c", c=HQ))

    nc.compile()
    return nc


def _build_and_jit():
    import jax
    from jax.sharding import Mesh, PartitionSpec
    from jax.experimental.shard_map import shard_map
    from concourse import bass2jax

    nc = _build_nc()

    bass2jax.install_neuronx_cc_hook()
    import concourse.mybir as mb

    partition_name = (nc.partition_id_tensor.name
                      if nc.partition_id_tensor else None)
    in_names, out_names, out_avals, zero_outs = [], [], [], []
    for alloc in nc.m.functions[0].allocations:
        if not isinstance(alloc, mb.MemoryLocationSet):
            continue
        name = alloc.memorylocations[0].name
        if alloc.kind == "ExternalInput":
            if name != partition_name:
                in_names.append(name)
        elif alloc.kind == "ExternalOutput":
            shape = tuple(alloc.tensor_shape)
            dtype = mb.dt.np(alloc.dtype)
            out_names.append(name)
            out_avals.append(jax.core.ShapedArray(shape, dtype))
            zero_outs.append(np.zeros(shape, dtype))
    n_params = len(in_names)
    n_outs = len(out_avals)
    in_names_all = list(in_names) + list(out_names)
    if partition_name is not None:
        in_names_all.append(partition_name)

    def _body(*args):
        operands = list(args)
        if partition_name is not None:
            operands.append(bass2jax.partition_id_tensor())
        outs = bass2jax._bass_exec_p.bind(
            *operands,
            out_avals=tuple(out_avals),
            in_names=tuple(in_names_all),
            out_names=tuple(out_names),
            lowering_input_output_aliases=(),
            sim_require_finite=True,
            sim_require_nnan=True,
            nc=nc,
        )
        return tuple(outs)

    devices = jax.devices()[:NCORES]
    mesh = Mesh(np.asarray(devices), ("core",))
    in_specs = (PartitionSpec("core"),) * (n_params + n_outs)
    out_specs = (PartitionSpec("core"),) * n_outs
    donate = tuple(range(n_params, n_params + n_outs))
    sharded = jax.jit(
        shard_map(_body, mesh=mesh, in_specs=in_specs,
                  out_specs=out_specs, check_rep=False),
        donate_argnums=donate, keep_unused=True)

    _STATE.update(dict(
        nc=nc, sharded=sharded, in_names=in_names, out_names=out_names,
        out_avals=out_avals, zero_outs=zero_outs, mesh=mesh))
    return _STATE


def _runner():
    if "sharded" not in _STATE:
        _build_and_jit()
    return _STATE


def make_in_maps(token_inputs, w_router, w1, w2):
    """Per-core input dicts (host-side shard/layout/dtype prep only)."""
    bf = ml_dtypes.bfloat16
    ones_c = np.ones((P, P), np.float32)
    utri_c = np.triu(np.ones((P, P), np.float32))
    iota64 = np.tile(np.arange(E, dtype=np.float32), (P, 8))
    siota = np.tile(np.arange(512, dtype=np.float32), (P, 1))
    consts = np.concatenate([ones_c, utri_c, iota64, siota],
                            axis=1).astype(np.float32)
    wr_p = np.ascontiguousarray(
        w_router.astype(np.float32).reshape(8, P, E)
        .transpose(1, 0, 2).reshape(P, 8 * E))
    in_maps = []
    for g in range(NCORES):
        tg = token_inputs[g].astype(np.float32)          # [T, H]
        tokT = tg.T.reshape(8, P, T)                     # [k, p, t]
        tok_t1 = tokT[:, :, :512].transpose(1, 0, 2).reshape(P, 8 * 512)
        tok_t2 = tokT[:, :, 512:].transpose(1, 0, 2).reshape(P, 8 * 512)
        tok_bp = (tg.astype(bf).reshape(NT, P, H)
                  .transpose(1, 0, 2).reshape(P, NT * H))
        # w1 [H, DFF] -> [p, (mb k c)]: lhsT tiles [p, k(h-tile), c(dff)]
        w1g = w1[g].astype(bf).reshape(8, P, 8, 512)      # [k, p, mb, c]
        w1_p = (w1g.transpose(1, 2, 0, 3)                 # [p, mb, k, c]
                .reshape(P, 8 * DFF))
        # w2 [DFF, H] -> [p, (hq kb k c)]: rhs tiles [p(dff), k, c(h)]
        w2g = w2[g].astype(bf).reshape(4, 8, P, 4, 256)   # [kb, k, p, hq, c]
        w2_p = (w2g.transpose(2, 3, 0, 1, 4)              # [p, hq, kb, k, c]
                .reshape(P, 8 * DFF))
        in_maps.append({
            "tok_t1": np.ascontiguousarray(tok_t1),
            "tok_t2": np.ascontiguousarray(tok_t2),
            "tok_bp": np.ascontiguousarray(tok_bp),
            "wrp": wr_p,
            "w1p": np.ascontiguousarray(w1_p),
            "w2p": np.ascontiguousarray(w2_p),
            "consts": consts,
        })
    return in_maps


def run_in_maps(in_maps):
    st = _runner()
    concat_in = [
        np.concatenate([np.asarray(in_maps[c][name])
                        for c in range(NCORES)], axis=0)
        for name in st["in_names"]
    ]
    concat_zeros = [np.zeros((NCORES * z.shape[0], *z.shape[1:]), z.dtype)
                    for z in st["zero_outs"]]
    out_arrs = st["sharded"](*concat_in, *concat_zeros)
    res = []
    for c in range(NCORES):
        res.append({
            name: np.asarray(out_arrs[i]).reshape(
                NCORES, *st["out_avals"][i].shape)[c]
            for i, name in enumerate(st["out_names"])
        })
    return res


def kernel(token_inputs, w_router, w1, w2, expert_capacity):
    token_inputs = np.asarray(token_inputs)
    w_router = np.asarray(w_router)
    w1 = np.asarray(w1)
    w2 = np.asarray(w2)
    assert int(expert_capacity) == CAP
    assert token_inputs.shape == (G, T, H)
    in_maps = make_in_maps(token_inputs, w_router, w1, w2)
    try:
        res = run_in_maps(in_maps)
    except Exception:
        from concourse import bass_utils
        nc = _STATE.get("nc") or _build_nc()
        res = bass_utils.run_bass_kernel_spmd(
            nc, in_maps, core_ids=list(range(NCORES))).results
    return np.stack([res[g]["out"] for g in range(NCORES)], axis=0)


# revision 3
# speedup vs baseline: 1.0344x; 1.0344x over previous
"""MoE routing + expert FFN kernel for 8 Trainium2 NeuronCores — v2.

Sharding: data-parallel routing (core g owns token group g) + expert-parallel
FFN (core e owns expert e); dispatch/combine are on-device AllToAlls.

v2 restructure vs v1 (362.7us cost model):
  - Host pre-packs every streamed tensor into the exact SBUF layout so each
    load is ONE contiguous whole-partition-line DMA (descriptor-gen and
    DGE-queue time dominated the v1 head).
  - Dispatch matmul emits the TRANSPOSED layout xdispT[e, h, c] directly
    (lhsT = token tile, moving = one-hot slot mask); the post-A2A xbar
    transpose disappears. A2A#1 is split into cap-halves so M1 starts
    after the first half lands.
  - M2/A2A#2/combine split into four h-quarters: each quarter's collective,
    gather, scale and store hide under the next quarter's matmuls.
  - yy/ycomb in bf16; batched combine gather (one indirect DMA per quarter).
  - Queues: SP carries token/w1/w2/xt streams; ACT carries consts, xdispT,
    yy and out writes; gpsimd carries collectives + gathers.
"""

import sys

sys.path.insert(0, "/opt/trn_rl_repo")

import numpy as np
import ml_dtypes

G, T, H, E, DFF, CAP = 8, 1024, 1024, 8, 4096, 128
NCORES = 8
P = 128
CH = CAP // 2  # cap half per A2A#1 piece
HQ = H // 4    # h quarter per A2A#2 piece
NT = T // P    # 8 token tiles per group

_STATE = {}
DISPATCH_MODE = "pe"  # scatter_batched|scatter_tile|pe|hybrid
BATCHED_GATHER = False


def _build_nc(fake_collectives=False, stages=None, dispatch_mode=None,
              batched_gather=None):
    if dispatch_mode is None:
        dispatch_mode = DISPATCH_MODE
    if batched_gather is None:
        batched_gather = BATCHED_GATHER
    from concourse import bacc
    import concourse.bass as bass
    import concourse.mybir as mybir
    import concourse.tile as tile

    f32 = mybir.dt.float32
    bf16 = mybir.dt.bfloat16
    i32 = mybir.dt.int32
    X = mybir.AxisListType.X
    AF = mybir.ActivationFunctionType
    OP = mybir.AluOpType

    nc = bacc.Bacc("TRN2", target_bir_lowering=False, debug=False,
                   num_devices=NCORES)

    # host-prepacked inputs (already in SBUF layout, partition dim first)
    tok_t1 = nc.dram_tensor("tok_t1", [P, 8 * 512], f32, kind="ExternalInput")
    tok_t2 = nc.dram_tensor("tok_t2", [P, 8 * 512], f32, kind="ExternalInput")
    tok_bp = nc.dram_tensor("tok_bp", [P, NT * H], bf16, kind="ExternalInput")
    wrp = nc.dram_tensor("wrp", [P, 8 * E], f32, kind="ExternalInput")
    w1p_d = nc.dram_tensor("w1p", [P, 8 * DFF], bf16, kind="ExternalInput")
    w2p_d = nc.dram_tensor("w2p", [P, 8 * DFF], bf16, kind="ExternalInput")
    consts = nc.dram_tensor("consts", [P, 2 * P + 8 * E + 512], f32,
                            kind="ExternalInput")
    out = nc.dram_tensor("out", [T, H], f32, kind="ExternalOutput")

    # A2A#1 carried as [E*CH, H] cap-halves (rows e*CH+c, +1 dump row);
    # A2A#2 as [T(+1), HQ] h-quarters.
    if dispatch_mode in ("pe", "hybrid"):
        npe = 2 if dispatch_mode == "pe" else 1
        xdispT = [[nc.dram_tensor(f"xdispT{i}_{j}", [E, H // 4, CH], bf16)
                   for j in range(4)] for i in range(npe)]
        xrecvT = [[nc.dram_tensor(f"xrecvT{i}_{j}", [G, H // 4, CH], bf16)
                   for j in range(4)] for i in range(npe)]
    if dispatch_mode != "pe":
        xdisp = [nc.dram_tensor(f"xdisp{i}", [E * CH + 1, H], bf16)
                 for i in range(2)]
        xrecv = [nc.dram_tensor(f"xrecv{i}", [G * CH, H], bf16)
                 for i in range(2)]
    yy = [nc.dram_tensor(f"yy{i}", [T, HQ], bf16) for i in range(4)]
    ycomb = [nc.dram_tensor(f"ycomb{i}", [T + 1, HQ], bf16) for i in range(4)]

    RG = [list(range(NCORES))]
    ALL = {"router", "cumsum", "dispatch", "m1", "m2", "combine"}
    stg = ALL if stages is None else set(stages)

    def _n(stage, n):
        return n if stage in stg else 0

    def a2a(in_t, out_t):
        if fake_collectives:
            nc.gpsimd.dma_start(out=out_t, in_=in_t)
        else:
            nc.gpsimd.collective_compute(
                "AllToAll", mybir.AluOpType.bypass, replica_groups=RG,
                ins=[in_t.opt()], outs=[out_t.opt()])

    with tile.TileContext(nc) as tc:
        with (
            tc.tile_pool(name="const", bufs=1) as constp,
            tc.tile_pool(name="big", bufs=1) as big,
            tc.tile_pool(name="rt", bufs=1) as rtp,
            tc.tile_pool(name="w1s_p", bufs=3) as w1pool,
            tc.tile_pool(name="w2s_p", bufs=3) as w2pool,
            tc.tile_pool(name="io", bufs=3) as iop,
            tc.tile_pool(name="cbp", bufs=2) as cbp,
            tc.tile_pool(name="ps1", bufs=3, space="PSUM") as ps1,
            tc.tile_pool(name="ps2", bufs=1, space="PSUM") as ps2,
        ):
            # ---- small consts first (they gate router/meta), then tokens on
            # both queues; tokb (needed only at dispatch) trails on SP
            tokT_sb = big.tile([P, 8 * T], f32)
            tokb_sb = big.tile([P, NT * H], bf16)
            wr_sb = constp.tile([P, 8 * E], f32)
            nc.scalar.dma_start(wr_sb[:], wrp[:, :])
            call = constp.tile([P, 2 * P + 8 * E + 512], f32)
            nc.scalar.dma_start(call[:], consts[:, :])
            nc.sync.dma_start(
                tokT_sb[:].rearrange("p (k t) -> p k t", k=8)[:, :, :512],
                tok_t1[:, :].rearrange("p (k t) -> p k t", k=8))
            nc.sync.dma_start(
                tokT_sb[:].rearrange("p (k t) -> p k t", k=8)[:, :, 512:],
                tok_t2[:, :].rearrange("p (k t) -> p k t", k=8))
            nc.sync.dma_start(tokb_sb[:], tok_bp[:, :])

            ones_sb = constp.tile([P, P], bf16)
            nc.vector.tensor_copy(ones_sb[:], call[:, 0:P])
            utri_sb = constp.tile([P, P], bf16)
            nc.vector.tensor_copy(utri_sb[:], call[:, P:2 * P])
            iota_sb = call[:, 2 * P:2 * P + 8 * E]
            siota_sb = call[:, 2 * P + 8 * E:]
            zrow = constp.tile([1, HQ], bf16)
            nc.vector.memset(zrow[:], 0.0)
            for i in range(4):
                nc.scalar.dma_start(ycomb[i][T:T + 1, :], zrow[:, :])

            # ---- router: per-token-tile k-chains (psr bufs=2 pingpong)
            lg = rtp.tile([P, NT * E], f32)
            for m in range(_n("router", NT)):
                lg_ps = ps1.tile([P, E], f32, name="lg_ps", tag="hps")
                for k in range(8):
                    nc.tensor.matmul(
                        lg_ps[:],
                        lhsT=tokT_sb[:, k * T + m * P: k * T + (m + 1) * P],
                        rhs=wr_sb[:, k * E:(k + 1) * E],
                        start=(k == 0), stop=(k == 7))
                nc.vector.tensor_copy(lg[:, m * E:(m + 1) * E], lg_ps[:])

            maskb = big.tile([P, NT * E], bf16)
            maskf_all = big.tile([P, NT * E], f32)
            gate_all = big.tile([P, NT], f32)
            idx_all = big.tile([P, NT], f32)
            addr_i = big.tile([P, NT], i32)
            scale_all = big.tile([P, NT], f32)
            addr_f = big.tile([P, NT], f32)
            addr_hf = [big.tile([P, NT], f32, name=f"addr_hf{i}")
                       for i in range(2)]
            addr_hi = [big.tile([P, NT], i32, name=f"addr_hi{i}")
                       for i in range(2)]

            dmask = big.tile([P, NT * T], bf16)
            _dmask_early = [False]

            def build_dmask(half):
                for m in range(_n("dispatch", NT)):
                    nc.vector.tensor_scalar(
                        dmask[:, m * T + half * 512:
                              m * T + (half + 1) * 512],
                        siota_sb, addr_hf[half][:, m:m + 1],
                        None, op0=OP.is_equal)

            def build_dmask_h0():
                build_dmask(0)

            if "router" in stg:
                mx = rtp.tile([P, NT], f32)
                nc.vector.tensor_reduce(
                    mx[:], lg[:].rearrange("p (m e) -> p m e", e=E), axis=X,
                    op=OP.max)
                mxb = mx[:].rearrange("p m -> p m ()").broadcast_to([P, NT, E])
                lg3 = lg[:].rearrange("p (m e) -> p m e", e=E)
                # mask = (logit >= rowmax), as f32 and bf16
                nc.vector.tensor_tensor(
                    maskf_all[:].rearrange("p (m e) -> p m e", e=E),
                    lg3, mxb, op=OP.is_ge)
                nc.vector.tensor_copy(maskb[:], maskf_all[:])
                # exp(logit - rowmax), summed over e -> 1/gate
                exm = rtp.tile([P, NT * E], f32)
                nc.vector.tensor_tensor(
                    exm[:].rearrange("p (m e) -> p m e", e=E),
                    lg3, mxb, op=OP.subtract)
                ex = rtp.tile([P, NT * E], f32)
                nc.scalar.activation(ex[:], exm[:], AF.Exp)
                esum = rtp.tile([P, NT], f32)
                nc.vector.reduce_sum(
                    esum[:], ex[:].rearrange("p (m e) -> p m e", e=E), axis=X)
                nc.vector.reciprocal(gate_all[:], esum[:])
                # expert index = sum(mask * iota)
                iw = rtp.tile([P, NT * E], f32)
                nc.vector.tensor_tensor(iw[:], maskf_all[:], iota_sb,
                                        op=OP.mult)
                nc.vector.reduce_sum(
                    idx_all[:], iw[:].rearrange("p (m e) -> p m e", e=E),
                    axis=X)

            # ---- capacity positions: utri within tile + running col sums
            cum_all = big.tile([P, NT * E], f32)
            for m in range(_n("cumsum", NT)):
                cum_ps = ps1.tile([P, E], f32, name="cum_ps", tag="hps")
                for k in range(m + 1):
                    nc.tensor.matmul(
                        cum_ps[:],
                        lhsT=(utri_sb[:] if k == m else ones_sb[:]),
                        rhs=maskb[:, k * E:(k + 1) * E],
                        start=(k == 0), stop=(k == m))
                nc.vector.tensor_copy(cum_all[:, m * E:(m + 1) * E], cum_ps[:])
            if "cumsum" in stg:
                mcum = rtp.tile([P, NT * E], f32)
                nc.vector.tensor_tensor(mcum[:], maskf_all[:], cum_all[:],
                                        op=OP.mult)
                pos = rtp.tile([P, NT], f32)
                nc.vector.reduce_sum(
                    pos[:], mcum[:].rearrange("p (m e) -> p m e", e=E), axis=X)
                nc.vector.tensor_scalar_sub(pos[:], pos[:], 1.0)
                kept = rtp.tile([P, NT], f32)
                nc.vector.tensor_scalar(kept[:], pos[:], float(CAP), None,
                                        op0=OP.is_lt)
                # per-cap-half addresses FIRST (they gate dmask/dispatch):
                # addr_h = (idx*CH + pos - half*CH) if half owns pos else 512
                ish = rtp.tile([P, NT], f32, name="ish")
                base = [rtp.tile([P, NT], f32, name=f"base{i}")
                        for i in range(2)]
                for half in range(2):
                    if half == 0:
                        nc.vector.tensor_scalar(ish[:], pos[:], float(CH),
                                                None, op0=OP.is_lt)
                    else:
                        nc.vector.tensor_scalar(ish[:], pos[:], float(CH),
                                                None, op0=OP.is_ge)
                        nc.vector.tensor_tensor(ish[:], ish[:], kept[:],
                                                op=OP.mult)
                    nc.vector.tensor_scalar_mul(base[half][:], idx_all[:],
                                                float(CH))
                    nc.vector.tensor_tensor(base[half][:], base[half][:],
                                            pos[:], op=OP.add)
                    nc.vector.tensor_scalar_sub(
                        base[half][:], base[half][:], float(half * CH + 512))
                    nc.vector.tensor_tensor(base[half][:], base[half][:],
                                            ish[:], op=OP.mult)
                    nc.vector.tensor_scalar(addr_hf[half][:], base[half][:],
                                            512.0, 0.0, op0=OP.add,
                                            op1=OP.max)
                if dispatch_mode in ("pe", "hybrid") and "dispatch" in stg:
                    _dmask_early[0] = True
                    build_dmask_h0()
                # combine-side addresses + gate scale (needed much later)
                drop = rtp.tile([P, NT], f32)
                nc.vector.tensor_scalar(drop[:], pos[:], float(CAP), None,
                                        op0=OP.is_ge)
                nc.vector.tensor_scalar_mul(addr_f[:], idx_all[:], float(CAP))
                nc.vector.tensor_tensor(addr_f[:], addr_f[:], pos[:],
                                        op=OP.add)
                nc.vector.tensor_tensor(addr_f[:], addr_f[:], kept[:],
                                        op=OP.mult)
                nc.vector.tensor_scalar_mul(drop[:], drop[:], float(T))
                nc.vector.tensor_tensor(addr_f[:], addr_f[:], drop[:],
                                        op=OP.add)
                nc.vector.tensor_scalar_max(addr_f[:], addr_f[:], 0.0)
                nc.vector.tensor_scalar_min(addr_f[:], addr_f[:], float(T))
                nc.vector.tensor_copy(addr_i[:], addr_f[:])
                nc.vector.tensor_tensor(scale_all[:], gate_all[:], kept[:],
                                        op=OP.mult)
                for half in range(2):
                    nc.vector.tensor_copy(addr_hi[half][:], addr_hf[half][:])

            # ---- dispatch per cap-half: either an indirect row-scatter
            # (token rows -> slot rows e*CH+c, dropped -> dump row 512), or a
            # PE one-hot matmul emitting the transposed layout directly
            if dispatch_mode in ("pe", "hybrid"):
                if not _dmask_early[0]:
                    build_dmask_h0()
            for half in range(_n("dispatch", 2)):
                if dispatch_mode == "hybrid":
                    if half == 0:
                        for hb in range(8):
                            dps = ps1.tile([P, 512], f32, name="dps",
                                           tag="hps")
                            for tb in range(NT):
                                nc.tensor.matmul(
                                    dps[:],
                                    lhsT=tokb_sb[:, tb * H + hb * P:
                                                 tb * H + (hb + 1) * P],
                                    rhs=dmask[:, tb * T:tb * T + 512],
                                    start=(tb == 0), stop=(tb == NT - 1))
                            xo = iop.tile([P, 512], bf16, name="xo",
                                          tag="xo")
                            nc.scalar.activation(xo[:], dps[:], AF.Copy)
                            nc.scalar.dma_start(
                                xdispT[0][hb // 4]
                                [:, (hb % 4) * P:(hb % 4 + 1) * P, :]
                                .transpose([1, 0, 2]),
                                xo[:].rearrange("p (e c) -> p e c", c=CH))
                            if hb == 3:
                                a2a(xdispT[0][0][:, :, :],
                                    xrecvT[0][0][:, :, :])
                        a2a(xdispT[0][1][:, :, :], xrecvT[0][1][:, :, :])
                    else:
                        # half-B row scatter runs on the Pool queue behind
                        # the half-A collectives, hidden under M1-A
                        for m in range(NT):
                            nc.gpsimd.indirect_dma_start(
                                out=xdisp[1][:, :],
                                out_offset=bass.IndirectOffsetOnAxis(
                                    ap=addr_hi[1][:, m:m + 1], axis=0),
                                in_=tokb_sb[:, m * H:(m + 1) * H],
                                in_offset=None)
                        a2a(xdisp[1][0:E * CH, :], xrecv[1][:, :])
                elif dispatch_mode == "pe":
                    for hb in range(8):
                        dps = ps1.tile([P, 512], f32, name="dps", tag="hps")
                        for tb in range(NT):
                            nc.tensor.matmul(
                                dps[:],
                                lhsT=tokb_sb[:, tb * H + hb * P:
                                             tb * H + (hb + 1) * P],
                                rhs=dmask[:, tb * T + half * 512:
                                          tb * T + (half + 1) * 512],
                                start=(tb == 0), stop=(tb == NT - 1))
                        xo = iop.tile([P, 512], bf16, name="xo", tag="xo")
                        nc.scalar.activation(xo[:], dps[:], AF.Copy)
                        nc.scalar.dma_start(
                            xdispT[half][hb // 2]
                            [:, (hb % 2) * P:(hb % 2 + 1) * P, :]
                            .transpose([1, 0, 2]),
                            xo[:].rearrange("p (e c) -> p e c", c=CH))
                        if hb == 0 and half == 0:
                            build_dmask(1)
                        if hb % 2 == 1:
                            a2a(xdispT[half][hb // 2][:, :, :],
                                xrecvT[half][hb // 2][:, :, :])
                elif dispatch_mode == "scatter_batched":
                    nc.gpsimd.indirect_dma_start(
                        out=xdisp[half][:, :],
                        out_offset=bass.IndirectOffsetOnAxis(
                            ap=addr_hi[half][:, :], axis=0),
                        in_=tokb_sb[:].rearrange("p (m h) -> p m h", h=H),
                        in_offset=None)
                    a2a(xdisp[half][0:E * CH, :], xrecv[half][:, :])
                else:
                    for m in range(NT):
                        nc.gpsimd.indirect_dma_start(
                            out=xdisp[half][:, :],
                            out_offset=bass.IndirectOffsetOnAxis(
                                ap=addr_hi[half][:, m:m + 1], axis=0),
                            in_=tokb_sb[:, m * H:(m + 1) * H],
                            in_offset=None)
                    a2a(xdisp[half][0:E * CH, :], xrecv[half][:, :])

            # ---- M1: hT[dff, slot] = relu(w1.T @ x) per cap-half
            # slot columns within ht_sb: (g, c) with c global (0..127)
            # w1s loads software-pipelined 2 deep ahead of the compute
            ht_sb = big.tile([P, 32 * T], bf16)
            w1s_tiles = {}

            def load_w1(mb):
                t = w1pool.tile([P, 8 * 512], bf16, name="w1s")
                nc.sync.dma_start(t[:], w1p_d[:, mb * 4096:(mb + 1) * 4096])
                return t

            nw1 = _n("m1", 2) * 8
            xt_sbs = [big.tile([P, 8 * 512], bf16, name=f"xt_sb{i}")
                      for i in range(2)]

            def stage_xt(half):
                for k in range(8):
                    if dispatch_mode == "hybrid":
                        if half == 0:
                            nc.sync.dma_start(
                                xt_sbs[0][:, k * 512:(k + 1) * 512]
                                .rearrange("p (g c) -> p g c", c=CH),
                                xrecvT[0][k // 4]
                                [:, (k % 4) * P:(k % 4 + 1) * P, :]
                                .transpose([1, 0, 2]))
                        else:
                            nc.sync.dma_start_transpose(
                                xt_sbs[1][:, k * 512:(k + 1) * 512],
                                xrecv[1][:, k * P:(k + 1) * P])
                    elif dispatch_mode == "pe":
                        nc.sync.dma_start(
                            xt_sbs[half][:, k * 512:(k + 1) * 512]
                            .rearrange("p (g c) -> p g c", c=CH),
                            xrecvT[half][k // 2]
                            [:, (k % 2) * P:(k % 2 + 1) * P, :]
                            .transpose([1, 0, 2]))
                    else:
                        nc.sync.dma_start_transpose(
                            xt_sbs[half][:, k * 512:(k + 1) * 512],
                            xrecv[half][:, k * P:(k + 1) * P])

            if _n("m1", 2):
                stage_xt(0)
            for half in range(_n("m1", 2)):
                xt_sb = xt_sbs[half]
                for mb in range(8):
                    if half == 0 and mb == 4:
                        stage_xt(1)
                    j = half * 8 + mb
                    if j == 0:
                        for jj in range(min(2, nw1)):
                            w1s_tiles[jj] = load_w1(jj % 8)
                    w1s = w1s_tiles.pop(j)
                    if j + 2 < nw1:
                        w1s_tiles[j + 2] = load_w1((j + 2) % 8)
                    for m4 in range(4):
                        mm = mb * 4 + m4
                        hps = ps1.tile([P, 512], f32, name="hps", tag="hps")
                        for k in range(8):
                            nc.tensor.matmul(
                                hps[:],
                                lhsT=w1s[:, k * 512 + m4 * P:
                                         k * 512 + (m4 + 1) * P],
                                rhs=xt_sb[:, k * 512:(k + 1) * 512],
                                start=(k == 0), stop=(k == 7))
                        nc.scalar.activation(
                            ht_sb[:, mm * T:(mm + 1) * T]
                            .rearrange("p (g c) -> p g c", c=CAP)
                            [:, :, half * CH:(half + 1) * CH],
                            hps[:], AF.Relu)

            # ---- M2: yy[slot, h] = hT.T @ w2 per h-quarter; slot tile = group
            for hq in range(_n("m2", 4)):
                for tmb in range(2):
                    pss = [ps2.tile([P, 256], f32, name=f"pss{i}",
                                    tag=f"pss{i}", bufs=1)
                           for i in range(4)]
                    for kb in range(4):
                        w2s = w2pool.tile([P, 8 * 256], bf16)
                        with tc.tile_wait_until(
                                0.040, enable=(hq == 0 and tmb == 0
                                               and kb < 2)):
                            nc.sync.dma_start(
                                w2s[:], w2p_d[:, (hq * 4 + kb) * 2048:
                                              (hq * 4 + kb + 1) * 2048])
                        for t4 in range(4):
                            tm = tmb * 4 + t4
                            for k in range(8):
                                kk = kb * 8 + k
                                nc.tensor.matmul(
                                    pss[t4][:],
                                    lhsT=ht_sb[:, kk * T + tm * P:
                                               kk * T + (tm + 1) * P],
                                    rhs=w2s[:, k * 256:(k + 1) * 256],
                                    start=(kk == 0), stop=(kk == 31))
                    for t4 in range(4):
                        tm = tmb * 4 + t4
                        yo = iop.tile([P, 256], bf16, name="yo", tag="yo")
                        if t4 % 2 == 0:
                            nc.vector.tensor_copy(yo[:], pss[t4][:])
                        else:
                            nc.scalar.activation(yo[:], pss[t4][:], AF.Copy)
                        nc.scalar.dma_start(yy[hq][tm * P:(tm + 1) * P, :],
                                            yo[:])
                # ---- A2A#2 + combine gather for this h-quarter
                if "m2" in stg:
                    a2a(yy[hq][:, :], ycomb[hq][0:T, :])
                if "combine" in stg:
                    # gather y rows by token address; pipeline gathers with
                    # the scale+store chunks, writes on alternating queues
                    cb = cbp.tile([P, NT * HQ], bf16, name="cb", tag="cb",
                                  bufs=2)
                    if batched_gather:
                        nc.gpsimd.indirect_dma_start(
                            out=cb[:].rearrange("p (m c) -> p m c", c=HQ),
                            out_offset=None,
                            in_=ycomb[hq][:, :],
                            in_offset=bass.IndirectOffsetOnAxis(
                                ap=addr_i[:, :], axis=0))
                    for q in range(4):
                        m0 = q * 2
                        if not batched_gather:
                            for m in range(m0, m0 + 2):
                                nc.gpsimd.indirect_dma_start(
                                    out=cb[:, m * HQ:(m + 1) * HQ],
                                    out_offset=None,
                                    in_=ycomb[hq][:, :],
                                    in_offset=bass.IndirectOffsetOnAxis(
                                        ap=addr_i[:, m:m + 1], axis=0))
                        cf = cbp.tile([P, 2 * HQ], f32, name="cf", tag="cf",
                                      bufs=2)
                        nc.vector.tensor_tensor(
                            cf[:].rearrange("p (m c) -> p m c", c=HQ),
                            cb[:, m0 * HQ:(m0 + 2) * HQ]
                            .rearrange("p (m c) -> p m c", c=HQ),
                            scale_all[:, m0:m0 + 2]
                            .rearrange("p m -> p m ()")
                            .broadcast_to([P, 2, HQ]),
                            op=OP.mult)
                        (nc.sync if q % 2 == 0 else nc.scalar).dma_start(
                            out[m0 * P:(m0 + 2) * P,
                                hq * HQ:(hq + 1) * HQ].rearrange(
                                "(m p) c -> p m c", p=P),
                            cf[:].rearrange("p (m c) -> p m c", c=HQ))

    nc.compile()
    return nc


def _build_and_jit():
    import jax
    from jax.sharding import Mesh, PartitionSpec
    from jax.experimental.shard_map import shard_map
    from concourse import bass2jax

    nc = _build_nc()

    bass2jax.install_neuronx_cc_hook()
    import concourse.mybir as mb

    partition_name = (nc.partition_id_tensor.name
                      if nc.partition_id_tensor else None)
    in_names, out_names, out_avals, zero_outs = [], [], [], []
    for alloc in nc.m.functions[0].allocations:
        if not isinstance(alloc, mb.MemoryLocationSet):
            continue
        name = alloc.memorylocations[0].name
        if alloc.kind == "ExternalInput":
            if name != partition_name:
                in_names.append(name)
        elif alloc.kind == "ExternalOutput":
            shape = tuple(alloc.tensor_shape)
            dtype = mb.dt.np(alloc.dtype)
            out_names.append(name)
            out_avals.append(jax.core.ShapedArray(shape, dtype))
            zero_outs.append(np.zeros(shape, dtype))
    n_params = len(in_names)
    n_outs = len(out_avals)
    in_names_all = list(in_names) + list(out_names)
    if partition_name is not None:
        in_names_all.append(partition_name)

    def _body(*args):
        operands = list(args)
        if partition_name is not None:
            operands.append(bass2jax.partition_id_tensor())
        outs = bass2jax._bass_exec_p.bind(
            *operands,
            out_avals=tuple(out_avals),
            in_names=tuple(in_names_all),
            out_names=tuple(out_names),
            lowering_input_output_aliases=(),
            sim_require_finite=True,
            sim_require_nnan=True,
            nc=nc,
        )
        return tuple(outs)

    devices = jax.devices()[:NCORES]
    mesh = Mesh(np.asarray(devices), ("core",))
    in_specs = (PartitionSpec("core"),) * (n_params + n_outs)
    out_specs = (PartitionSpec("core"),) * n_outs
    donate = tuple(range(n_params, n_params + n_outs))
    sharded = jax.jit(
        shard_map(_body, mesh=mesh, in_specs=in_specs,
                  out_specs=out_specs, check_rep=False),
        donate_argnums=donate, keep_unused=True)

    _STATE.update(dict(
        nc=nc, sharded=sharded, in_names=in_names, out_names=out_names,
        out_avals=out_avals, zero_outs=zero_outs, mesh=mesh))
    return _STATE


def _runner():
    if "sharded" not in _STATE:
        _build_and_jit()
    return _STATE


def make_in_maps(token_inputs, w_router, w1, w2):
    """Per-core input dicts (host-side shard/layout/dtype prep only)."""
    bf = ml_dtypes.bfloat16
    ones_c = np.ones((P, P), np.float32)
    utri_c = np.triu(np.ones((P, P), np.float32))
    iota64 = np.tile(np.arange(E, dtype=np.float32), (P, 8))
    siota = np.tile(np.arange(512, dtype=np.float32), (P, 1))
    consts = np.concatenate([ones_c, utri_c, iota64, siota],
                            axis=1).astype(np.float32)
    wr_p = np.ascontiguousarray(
        w_router.astype(np.float32).reshape(8, P, E)
        .transpose(1, 0, 2).reshape(P, 8 * E))
    in_maps = []
    for g in range(NCORES):
        tg = token_inputs[g].astype(np.float32)          # [T, H]
        tokT = tg.T.reshape(8, P, T)                     # [k, p, t]
        tok_t1 = tokT[:, :, :512].transpose(1, 0, 2).reshape(P, 8 * 512)
        tok_t2 = tokT[:, :, 512:].transpose(1, 0, 2).reshape(P, 8 * 512)
        tok_bp = (tg.astype(bf).reshape(NT, P, H)
                  .transpose(1, 0, 2).reshape(P, NT * H))
        # w1 [H, DFF] -> [p, (mb k c)]: lhsT tiles [p, k(h-tile), c(dff)]
        w1g = w1[g].astype(bf).reshape(8, P, 8, 512)      # [k, p, mb, c]
        w1_p = (w1g.transpose(1, 2, 0, 3)                 # [p, mb, k, c]
                .reshape(P, 8 * DFF))
        # w2 [DFF, H] -> [p, (hq kb k c)]: rhs tiles [p(dff), k, c(h)]
        w2g = w2[g].astype(bf).reshape(4, 8, P, 4, 256)   # [kb, k, p, hq, c]
        w2_p = (w2g.transpose(2, 3, 0, 1, 4)              # [p, hq, kb, k, c]
                .reshape(P, 8 * DFF))
        in_maps.append({
            "tok_t1": np.ascontiguousarray(tok_t1),
            "tok_t2": np.ascontiguousarray(tok_t2),
            "tok_bp": np.ascontiguousarray(tok_bp),
            "wrp": wr_p,
            "w1p": np.ascontiguousarray(w1_p),
            "w2p": np.ascontiguousarray(w2_p),
            "consts": consts,
        })
    return in_maps


def run_in_maps(in_maps):
    st = _runner()
    concat_in = [
        np.concatenate([np.asarray(in_maps[c][name])
                        for c in range(NCORES)], axis=0)
        for name in st["in_names"]
    ]
    concat_zeros = [np.zeros((NCORES * z.shape[0], *z.shape[1:]), z.dtype)
                    for z in st["zero_outs"]]
    out_arrs = st["sharded"](*concat_in, *concat_zeros)
    res = []
    for c in range(NCORES):
        res.append({
            name: np.asarray(out_arrs[i]).reshape(
                NCORES, *st["out_avals"][i].shape)[c]
            for i, name in enumerate(st["out_names"])
        })
    return res


def kernel(token_inputs, w_router, w1, w2, expert_capacity):
    token_inputs = np.asarray(token_inputs)
    w_router = np.asarray(w_router)
    w1 = np.asarray(w1)
    w2 = np.asarray(w2)
    assert int(expert_capacity) == CAP
    assert token_inputs.shape == (G, T, H)
    in_maps = make_in_maps(token_inputs, w_router, w1, w2)
    try:
        res = run_in_maps(in_maps)
    except Exception:
        from concourse import bass_utils
        nc = _STATE.get("nc") or _build_nc()
        res = bass_utils.run_bass_kernel_spmd(
            nc, in_maps, core_ids=list(range(NCORES))).results
    return np.stack([res[g]["out"] for g in range(NCORES)], axis=0)


# revision 4
# speedup vs baseline: 1.0389x; 1.0043x over previous
"""MoE routing + expert FFN kernel for 8 Trainium2 NeuronCores — v2.

Sharding: data-parallel routing (core g owns token group g) + expert-parallel
FFN (core e owns expert e); dispatch/combine are on-device AllToAlls.

v2 restructure vs v1 (362.7us cost model):
  - Host pre-packs every streamed tensor into the exact SBUF layout so each
    load is ONE contiguous whole-partition-line DMA (descriptor-gen and
    DGE-queue time dominated the v1 head).
  - Dispatch matmul emits the TRANSPOSED layout xdispT[e, h, c] directly
    (lhsT = token tile, moving = one-hot slot mask); the post-A2A xbar
    transpose disappears. A2A#1 fires per cap-half x h-tile (16 small
    pieces) so M1's k-chain consumes tiles as they land.
  - M2/A2A#2/combine split into four h-quarters; each quarter's collective,
    per-tile gathers, scale and store hide under the next quarter's matmuls
    (combine runs in 2-tile chunks, stores alternate SP/ACT queues).
  - yy/ycomb in bf16. Indirect DMAs use [P,1] offsets only: multi-offset
    forms crash or mis-address on HW (see scatter_* modes, kept for record).
  - Queues: SP carries token/w1/w2/xt streams (w1 x3, w2 x3 buffered); ACT
    carries consts, xdispT, yy and out writes; gpsimd collectives+gathers.
"""

import sys

sys.path.insert(0, "/opt/trn_rl_repo")

import numpy as np
import ml_dtypes

G, T, H, E, DFF, CAP = 8, 1024, 1024, 8, 4096, 128
NCORES = 8
P = 128
CH = CAP // 2  # cap half per A2A#1 piece
HQ = H // 4    # h quarter per A2A#2 piece
NT = T // P    # 8 token tiles per group

_STATE = {}
DISPATCH_MODE = "pe"  # scatter_batched|scatter_tile|pe|hybrid
BATCHED_GATHER = False


def _build_nc(fake_collectives=False, stages=None, dispatch_mode=None,
              batched_gather=None):
    if dispatch_mode is None:
        dispatch_mode = DISPATCH_MODE
    if batched_gather is None:
        batched_gather = BATCHED_GATHER
    from concourse import bacc
    import concourse.bass as bass
    import concourse.mybir as mybir
    import concourse.tile as tile

    f32 = mybir.dt.float32
    bf16 = mybir.dt.bfloat16
    i32 = mybir.dt.int32
    X = mybir.AxisListType.X
    AF = mybir.ActivationFunctionType
    OP = mybir.AluOpType

    nc = bacc.Bacc("TRN2", target_bir_lowering=False, debug=False,
                   num_devices=NCORES)

    # host-prepacked inputs (already in SBUF layout, partition dim first)
    tok_t1 = nc.dram_tensor("tok_t1", [P, 8 * 512], f32, kind="ExternalInput")
    tok_t2 = nc.dram_tensor("tok_t2", [P, 8 * 512], f32, kind="ExternalInput")
    tok_bp = nc.dram_tensor("tok_bp", [P, NT * H], bf16, kind="ExternalInput")
    wrp = nc.dram_tensor("wrp", [P, 8 * E], f32, kind="ExternalInput")
    w1p_d = nc.dram_tensor("w1p", [P, 8 * DFF], bf16, kind="ExternalInput")
    w2p_d = nc.dram_tensor("w2p", [P, 8 * DFF], bf16, kind="ExternalInput")
    consts = nc.dram_tensor("consts", [P, 2 * P + 8 * E + 512], f32,
                            kind="ExternalInput")
    out = nc.dram_tensor("out", [T, H], f32, kind="ExternalOutput")

    # A2A#1 carried as [E*CH, H] cap-halves (rows e*CH+c, +1 dump row);
    # A2A#2 as [T(+1), HQ] h-quarters.
    if dispatch_mode in ("pe", "hybrid"):
        npe = 2 if dispatch_mode == "pe" else 1
        xdispT = [[nc.dram_tensor(f"xdispT{i}_{j}", [E, H // 8, CH], bf16)
                   for j in range(8)] for i in range(npe)]
        xrecvT = [[nc.dram_tensor(f"xrecvT{i}_{j}", [G, H // 8, CH], bf16)
                   for j in range(8)] for i in range(npe)]
    if dispatch_mode != "pe":
        xdisp = [nc.dram_tensor(f"xdisp{i}", [E * CH + 1, H], bf16)
                 for i in range(2)]
        xrecv = [nc.dram_tensor(f"xrecv{i}", [G * CH, H], bf16)
                 for i in range(2)]
    yy = [nc.dram_tensor(f"yy{i}", [T, HQ], bf16) for i in range(4)]
    ycomb = [nc.dram_tensor(f"ycomb{i}", [T + 1, HQ], bf16) for i in range(4)]

    RG = [list(range(NCORES))]
    ALL = {"router", "cumsum", "dispatch", "m1", "m2", "combine"}
    stg = ALL if stages is None else set(stages)

    def _n(stage, n):
        return n if stage in stg else 0

    def a2a(in_t, out_t):
        if fake_collectives:
            nc.gpsimd.dma_start(out=out_t, in_=in_t)
        else:
            nc.gpsimd.collective_compute(
                "AllToAll", mybir.AluOpType.bypass, replica_groups=RG,
                ins=[in_t.opt()], outs=[out_t.opt()])

    with tile.TileContext(nc) as tc:
        with (
            tc.tile_pool(name="const", bufs=1) as constp,
            tc.tile_pool(name="big", bufs=1) as big,
            tc.tile_pool(name="rt", bufs=1) as rtp,
            tc.tile_pool(name="w1s_p", bufs=3) as w1pool,
            tc.tile_pool(name="w2s_p", bufs=3) as w2pool,
            tc.tile_pool(name="io", bufs=3) as iop,
            tc.tile_pool(name="cbp", bufs=2) as cbp,
            tc.tile_pool(name="ps1", bufs=3, space="PSUM") as ps1,
            tc.tile_pool(name="ps2", bufs=1, space="PSUM") as ps2,
        ):
            # ---- small consts first (they gate router/meta), then tokens on
            # both queues; tokb (needed only at dispatch) trails on SP
            tokT_sb = big.tile([P, 8 * T], f32)
            tokb_sb = big.tile([P, NT * H], bf16)
            wr_sb = constp.tile([P, 8 * E], f32)
            nc.scalar.dma_start(wr_sb[:], wrp[:, :])
            call = constp.tile([P, 2 * P + 8 * E + 512], f32)
            nc.scalar.dma_start(call[:], consts[:, :])
            nc.sync.dma_start(
                tokT_sb[:].rearrange("p (k t) -> p k t", k=8)[:, :, :512],
                tok_t1[:, :].rearrange("p (k t) -> p k t", k=8))
            nc.sync.dma_start(
                tokT_sb[:].rearrange("p (k t) -> p k t", k=8)[:, :, 512:],
                tok_t2[:, :].rearrange("p (k t) -> p k t", k=8))
            nc.sync.dma_start(tokb_sb[:], tok_bp[:, :])

            ones_sb = constp.tile([P, P], bf16)
            nc.vector.tensor_copy(ones_sb[:], call[:, 0:P])
            utri_sb = constp.tile([P, P], bf16)
            nc.vector.tensor_copy(utri_sb[:], call[:, P:2 * P])
            iota_sb = call[:, 2 * P:2 * P + 8 * E]
            siota_sb = call[:, 2 * P + 8 * E:]
            zrow = constp.tile([1, HQ], bf16)
            nc.vector.memset(zrow[:], 0.0)
            for i in range(4):
                nc.scalar.dma_start(ycomb[i][T:T + 1, :], zrow[:, :])

            # ---- router: per-token-tile k-chains (psr bufs=2 pingpong)
            lg = rtp.tile([P, NT * E], f32)
            for m in range(_n("router", NT)):
                lg_ps = ps1.tile([P, E], f32, name="lg_ps", tag="hps")
                for k in range(8):
                    nc.tensor.matmul(
                        lg_ps[:],
                        lhsT=tokT_sb[:, k * T + m * P: k * T + (m + 1) * P],
                        rhs=wr_sb[:, k * E:(k + 1) * E],
                        start=(k == 0), stop=(k == 7))
                nc.vector.tensor_copy(lg[:, m * E:(m + 1) * E], lg_ps[:])

            maskb = big.tile([P, NT * E], bf16)
            maskf_all = big.tile([P, NT * E], f32)
            gate_all = big.tile([P, NT], f32)
            idx_all = big.tile([P, NT], f32)
            addr_i = big.tile([P, NT], i32)
            scale_all = big.tile([P, NT], f32)
            addr_f = big.tile([P, NT], f32)
            addr_hf = [big.tile([P, NT], f32, name=f"addr_hf{i}")
                       for i in range(2)]
            addr_hi = [big.tile([P, NT], i32, name=f"addr_hi{i}")
                       for i in range(2)]

            dmask = big.tile([P, NT * T], bf16)
            _dmask_early = [False]

            def build_dmask(half):
                for m in range(_n("dispatch", NT)):
                    nc.vector.tensor_scalar(
                        dmask[:, m * T + half * 512:
                              m * T + (half + 1) * 512],
                        siota_sb, addr_hf[half][:, m:m + 1],
                        None, op0=OP.is_equal)

            def build_dmask_h0():
                build_dmask(0)

            if "router" in stg:
                mx = rtp.tile([P, NT], f32)
                nc.vector.tensor_reduce(
                    mx[:], lg[:].rearrange("p (m e) -> p m e", e=E), axis=X,
                    op=OP.max)
                mxb = mx[:].rearrange("p m -> p m ()").broadcast_to([P, NT, E])
                lg3 = lg[:].rearrange("p (m e) -> p m e", e=E)
                # mask = (logit >= rowmax), as f32 and bf16
                nc.vector.tensor_tensor(
                    maskf_all[:].rearrange("p (m e) -> p m e", e=E),
                    lg3, mxb, op=OP.is_ge)
                nc.vector.tensor_copy(maskb[:], maskf_all[:])
                # exp(logit - rowmax), summed over e -> 1/gate
                exm = rtp.tile([P, NT * E], f32)
                nc.vector.tensor_tensor(
                    exm[:].rearrange("p (m e) -> p m e", e=E),
                    lg3, mxb, op=OP.subtract)
                ex = rtp.tile([P, NT * E], f32)
                nc.scalar.activation(ex[:], exm[:], AF.Exp)
                esum = rtp.tile([P, NT], f32)
                nc.vector.reduce_sum(
                    esum[:], ex[:].rearrange("p (m e) -> p m e", e=E), axis=X)
                nc.vector.reciprocal(gate_all[:], esum[:])
                # expert index = sum(mask * iota)
                iw = rtp.tile([P, NT * E], f32)
                nc.vector.tensor_tensor(iw[:], maskf_all[:], iota_sb,
                                        op=OP.mult)
                nc.vector.reduce_sum(
                    idx_all[:], iw[:].rearrange("p (m e) -> p m e", e=E),
                    axis=X)

            # ---- capacity positions: utri within tile + running col sums
            cum_all = big.tile([P, NT * E], f32)
            for m in range(_n("cumsum", NT)):
                cum_ps = ps1.tile([P, E], f32, name="cum_ps", tag="hps")
                for k in range(m + 1):
                    nc.tensor.matmul(
                        cum_ps[:],
                        lhsT=(utri_sb[:] if k == m else ones_sb[:]),
                        rhs=maskb[:, k * E:(k + 1) * E],
                        start=(k == 0), stop=(k == m))
                nc.vector.tensor_copy(cum_all[:, m * E:(m + 1) * E], cum_ps[:])
            if "cumsum" in stg:
                mcum = rtp.tile([P, NT * E], f32)
                nc.vector.tensor_tensor(mcum[:], maskf_all[:], cum_all[:],
                                        op=OP.mult)
                pos = rtp.tile([P, NT], f32)
                nc.vector.reduce_sum(
                    pos[:], mcum[:].rearrange("p (m e) -> p m e", e=E), axis=X)
                nc.vector.tensor_scalar_sub(pos[:], pos[:], 1.0)
                kept = rtp.tile([P, NT], f32)
                nc.vector.tensor_scalar(kept[:], pos[:], float(CAP), None,
                                        op0=OP.is_lt)
                # per-cap-half addresses FIRST (they gate dmask/dispatch):
                # addr_h = (idx*CH + pos - half*CH) if half owns pos else 512
                ish = rtp.tile([P, NT], f32, name="ish")
                base = [rtp.tile([P, NT], f32, name=f"base{i}")
                        for i in range(2)]
                for half in range(2):
                    if half == 0:
                        nc.vector.tensor_scalar(ish[:], pos[:], float(CH),
                                                None, op0=OP.is_lt)
                    else:
                        nc.vector.tensor_scalar(ish[:], pos[:], float(CH),
                                                None, op0=OP.is_ge)
                        nc.vector.tensor_tensor(ish[:], ish[:], kept[:],
                                                op=OP.mult)
                    nc.vector.tensor_scalar_mul(base[half][:], idx_all[:],
                                                float(CH))
                    nc.vector.tensor_tensor(base[half][:], base[half][:],
                                            pos[:], op=OP.add)
                    nc.vector.tensor_scalar_sub(
                        base[half][:], base[half][:], float(half * CH + 512))
                    nc.vector.tensor_tensor(base[half][:], base[half][:],
                                            ish[:], op=OP.mult)
                    nc.vector.tensor_scalar(addr_hf[half][:], base[half][:],
                                            512.0, 0.0, op0=OP.add,
                                            op1=OP.max)
                if dispatch_mode in ("pe", "hybrid") and "dispatch" in stg:
                    _dmask_early[0] = True
                    build_dmask_h0()
                # combine-side addresses + gate scale (needed much later)
                drop = rtp.tile([P, NT], f32)
                nc.vector.tensor_scalar(drop[:], pos[:], float(CAP), None,
                                        op0=OP.is_ge)
                nc.vector.tensor_scalar_mul(addr_f[:], idx_all[:], float(CAP))
                nc.vector.tensor_tensor(addr_f[:], addr_f[:], pos[:],
                                        op=OP.add)
                nc.vector.tensor_tensor(addr_f[:], addr_f[:], kept[:],
                                        op=OP.mult)
                nc.vector.tensor_scalar_mul(drop[:], drop[:], float(T))
                nc.vector.tensor_tensor(addr_f[:], addr_f[:], drop[:],
                                        op=OP.add)
                nc.vector.tensor_scalar_max(addr_f[:], addr_f[:], 0.0)
                nc.vector.tensor_scalar_min(addr_f[:], addr_f[:], float(T))
                nc.vector.tensor_copy(addr_i[:], addr_f[:])
                nc.vector.tensor_tensor(scale_all[:], gate_all[:], kept[:],
                                        op=OP.mult)
                for half in range(2):
                    nc.vector.tensor_copy(addr_hi[half][:], addr_hf[half][:])

            # ---- dispatch per cap-half: either an indirect row-scatter
            # (token rows -> slot rows e*CH+c, dropped -> dump row 512), or a
            # PE one-hot matmul emitting the transposed layout directly
            if dispatch_mode in ("pe", "hybrid"):
                if not _dmask_early[0]:
                    build_dmask_h0()
            for half in range(_n("dispatch", 2)):
                if dispatch_mode == "hybrid":
                    if half == 0:
                        for hb in range(8):
                            dps = ps1.tile([P, 512], f32, name="dps",
                                           tag="hps")
                            for tb in range(NT):
                                nc.tensor.matmul(
                                    dps[:],
                                    lhsT=tokb_sb[:, tb * H + hb * P:
                                                 tb * H + (hb + 1) * P],
                                    rhs=dmask[:, tb * T:tb * T + 512],
                                    start=(tb == 0), stop=(tb == NT - 1))
                            xo = iop.tile([P, 512], bf16, name="xo",
                                          tag="xo")
                            nc.scalar.activation(xo[:], dps[:], AF.Copy)
                            nc.scalar.dma_start(
                                xdispT[0][hb // 4]
                                [:, (hb % 4) * P:(hb % 4 + 1) * P, :]
                                .transpose([1, 0, 2]),
                                xo[:].rearrange("p (e c) -> p e c", c=CH))
                            if hb == 3:
                                a2a(xdispT[0][0][:, :, :],
                                    xrecvT[0][0][:, :, :])
                        a2a(xdispT[0][1][:, :, :], xrecvT[0][1][:, :, :])
                    else:
                        # half-B row scatter runs on the Pool queue behind
                        # the half-A collectives, hidden under M1-A
                        for m in range(NT):
                            nc.gpsimd.indirect_dma_start(
                                out=xdisp[1][:, :],
                                out_offset=bass.IndirectOffsetOnAxis(
                                    ap=addr_hi[1][:, m:m + 1], axis=0),
                                in_=tokb_sb[:, m * H:(m + 1) * H],
                                in_offset=None)
                        a2a(xdisp[1][0:E * CH, :], xrecv[1][:, :])
                elif dispatch_mode == "pe":
                    for hb in range(8):
                        dps = ps1.tile([P, 512], f32, name="dps", tag="hps")
                        for tb in range(NT):
                            nc.tensor.matmul(
                                dps[:],
                                lhsT=tokb_sb[:, tb * H + hb * P:
                                             tb * H + (hb + 1) * P],
                                rhs=dmask[:, tb * T + half * 512:
                                          tb * T + (half + 1) * 512],
                                start=(tb == 0), stop=(tb == NT - 1))
                        xo = iop.tile([P, 512], bf16, name="xo", tag="xo")
                        nc.scalar.activation(xo[:], dps[:], AF.Copy)
                        nc.scalar.dma_start(
                            xdispT[half][hb][:, :, :]
                            .transpose([1, 0, 2]),
                            xo[:].rearrange("p (e c) -> p e c", c=CH))
                        if hb == 0 and half == 0:
                            build_dmask(1)
                        a2a(xdispT[half][hb][:, :, :],
                            xrecvT[half][hb][:, :, :])
                elif dispatch_mode == "scatter_batched":
                    nc.gpsimd.indirect_dma_start(
                        out=xdisp[half][:, :],
                        out_offset=bass.IndirectOffsetOnAxis(
                            ap=addr_hi[half][:, :], axis=0),
                        in_=tokb_sb[:].rearrange("p (m h) -> p m h", h=H),
                        in_offset=None)
                    a2a(xdisp[half][0:E * CH, :], xrecv[half][:, :])
                else:
                    for m in range(NT):
                        nc.gpsimd.indirect_dma_start(
                            out=xdisp[half][:, :],
                            out_offset=bass.IndirectOffsetOnAxis(
                                ap=addr_hi[half][:, m:m + 1], axis=0),
                            in_=tokb_sb[:, m * H:(m + 1) * H],
                            in_offset=None)
                    a2a(xdisp[half][0:E * CH, :], xrecv[half][:, :])

            # ---- M1: hT[dff, slot] = relu(w1.T @ x) per cap-half
            # slot columns within ht_sb: (g, c) with c global (0..127)
            # w1s loads software-pipelined 2 deep ahead of the compute
            ht_sb = big.tile([P, 32 * T], bf16)
            w1s_tiles = {}

            def load_w1(mb):
                t = w1pool.tile([P, 8 * 512], bf16, name="w1s")
                nc.sync.dma_start(t[:], w1p_d[:, mb * 4096:(mb + 1) * 4096])
                return t

            nw1 = _n("m1", 2) * 8
            xt_sbs = [big.tile([P, 8 * 512], bf16, name=f"xt_sb{i}")
                      for i in range(2)]

            def stage_xt(half):
                for k in range(8):
                    if dispatch_mode == "hybrid":
                        if half == 0:
                            nc.sync.dma_start(
                                xt_sbs[0][:, k * 512:(k + 1) * 512]
                                .rearrange("p (g c) -> p g c", c=CH),
                                xrecvT[0][k // 4]
                                [:, (k % 4) * P:(k % 4 + 1) * P, :]
                                .transpose([1, 0, 2]))
                        else:
                            nc.sync.dma_start_transpose(
                                xt_sbs[1][:, k * 512:(k + 1) * 512],
                                xrecv[1][:, k * P:(k + 1) * P])
                    elif dispatch_mode == "pe":
                        nc.sync.dma_start(
                            xt_sbs[half][:, k * 512:(k + 1) * 512]
                            .rearrange("p (g c) -> p g c", c=CH),
                            xrecvT[half][k][:, :, :]
                            .transpose([1, 0, 2]))
                    else:
                        nc.sync.dma_start_transpose(
                            xt_sbs[half][:, k * 512:(k + 1) * 512],
                            xrecv[half][:, k * P:(k + 1) * P])

            if _n("m1", 2):
                stage_xt(0)
            for half in range(_n("m1", 2)):
                xt_sb = xt_sbs[half]
                for mb in range(8):
                    if half == 0 and mb == 4:
                        stage_xt(1)
                    j = half * 8 + mb
                    if j == 0:
                        for jj in range(min(2, nw1)):
                            w1s_tiles[jj] = load_w1(jj % 8)
                    w1s = w1s_tiles.pop(j)
                    if j + 2 < nw1:
                        w1s_tiles[j + 2] = load_w1((j + 2) % 8)
                    for m4 in range(4):
                        mm = mb * 4 + m4
                        hps = ps1.tile([P, 512], f32, name="hps", tag="hps")
                        for k in range(8):
                            nc.tensor.matmul(
                                hps[:],
                                lhsT=w1s[:, k * 512 + m4 * P:
                                         k * 512 + (m4 + 1) * P],
                                rhs=xt_sb[:, k * 512:(k + 1) * 512],
                                start=(k == 0), stop=(k == 7))
                        nc.scalar.activation(
                            ht_sb[:, mm * T:(mm + 1) * T]
                            .rearrange("p (g c) -> p g c", c=CAP)
                            [:, :, half * CH:(half + 1) * CH],
                            hps[:], AF.Relu)

            # ---- M2: yy[slot, h] = hT.T @ w2 per h-quarter; slot tile = group
            for hq in range(_n("m2", 4)):
                for tmb in range(2):
                    pss = [ps2.tile([P, 256], f32, name=f"pss{i}",
                                    tag=f"pss{i}", bufs=1)
                           for i in range(4)]
                    for kb in range(4):
                        w2s = w2pool.tile([P, 8 * 256], bf16)
                        with tc.tile_wait_until(
                                0.040, enable=(hq == 0 and tmb == 0
                                               and kb < 2)):
                            nc.sync.dma_start(
                                w2s[:], w2p_d[:, (hq * 4 + kb) * 2048:
                                              (hq * 4 + kb + 1) * 2048])
                        for t4 in range(4):
                            tm = tmb * 4 + t4
                            for k in range(8):
                                kk = kb * 8 + k
                                nc.tensor.matmul(
                                    pss[t4][:],
                                    lhsT=ht_sb[:, kk * T + tm * P:
                                               kk * T + (tm + 1) * P],
                                    rhs=w2s[:, k * 256:(k + 1) * 256],
                                    start=(kk == 0), stop=(kk == 31))
                    for t4 in range(4):
                        tm = tmb * 4 + t4
                        yo = iop.tile([P, 256], bf16, name="yo", tag="yo")
                        if t4 % 2 == 0:
                            nc.vector.tensor_copy(yo[:], pss[t4][:])
                        else:
                            nc.scalar.activation(yo[:], pss[t4][:], AF.Copy)
                        nc.scalar.dma_start(yy[hq][tm * P:(tm + 1) * P, :],
                                            yo[:])
                # ---- A2A#2 + combine gather for this h-quarter
                if "m2" in stg:
                    a2a(yy[hq][:, :], ycomb[hq][0:T, :])
                if "combine" in stg:
                    # gather y rows by token address; pipeline gathers with
                    # the scale+store chunks, writes on alternating queues
                    cb = cbp.tile([P, NT * HQ], bf16, name="cb", tag="cb",
                                  bufs=2)
                    if batched_gather:
                        nc.gpsimd.indirect_dma_start(
                            out=cb[:].rearrange("p (m c) -> p m c", c=HQ),
                            out_offset=None,
                            in_=ycomb[hq][:, :],
                            in_offset=bass.IndirectOffsetOnAxis(
                                ap=addr_i[:, :], axis=0))
                    for q in range(4):
                        m0 = q * 2
                        if not batched_gather:
                            for m in range(m0, m0 + 2):
                                nc.gpsimd.indirect_dma_start(
                                    out=cb[:, m * HQ:(m + 1) * HQ],
                                    out_offset=None,
                                    in_=ycomb[hq][:, :],
                                    in_offset=bass.IndirectOffsetOnAxis(
                                        ap=addr_i[:, m:m + 1], axis=0))
                        cf = cbp.tile([P, 2 * HQ], f32, name="cf", tag="cf",
                                      bufs=2)
                        nc.vector.tensor_tensor(
                            cf[:].rearrange("p (m c) -> p m c", c=HQ),
                            cb[:, m0 * HQ:(m0 + 2) * HQ]
                            .rearrange("p (m c) -> p m c", c=HQ),
                            scale_all[:, m0:m0 + 2]
                            .rearrange("p m -> p m ()")
                            .broadcast_to([P, 2, HQ]),
                            op=OP.mult)
                        (nc.sync if q % 2 == 0 else nc.scalar).dma_start(
                            out[m0 * P:(m0 + 2) * P,
                                hq * HQ:(hq + 1) * HQ].rearrange(
                                "(m p) c -> p m c", p=P),
                            cf[:].rearrange("p (m c) -> p m c", c=HQ))

    nc.compile()
    return nc


def _build_and_jit():
    import jax
    from jax.sharding import Mesh, PartitionSpec
    from jax.experimental.shard_map import shard_map
    from concourse import bass2jax

    nc = _build_nc()

    bass2jax.install_neuronx_cc_hook()
    import concourse.mybir as mb

    partition_name = (nc.partition_id_tensor.name
                      if nc.partition_id_tensor else None)
    in_names, out_names, out_avals, zero_outs = [], [], [], []
    for alloc in nc.m.functions[0].allocations:
        if not isinstance(alloc, mb.MemoryLocationSet):
            continue
        name = alloc.memorylocations[0].name
        if alloc.kind == "ExternalInput":
            if name != partition_name:
                in_names.append(name)
        elif alloc.kind == "ExternalOutput":
            shape = tuple(alloc.tensor_shape)
            dtype = mb.dt.np(alloc.dtype)
            out_names.append(name)
            out_avals.append(jax.core.ShapedArray(shape, dtype))
            zero_outs.append(np.zeros(shape, dtype))
    n_params = len(in_names)
    n_outs = len(out_avals)
    in_names_all = list(in_names) + list(out_names)
    if partition_name is not None:
        in_names_all.append(partition_name)

    def _body(*args):
        operands = list(args)
        if partition_name is not None:
            operands.append(bass2jax.partition_id_tensor())
        outs = bass2jax._bass_exec_p.bind(
            *operands,
            out_avals=tuple(out_avals),
            in_names=tuple(in_names_all),
            out_names=tuple(out_names),
            lowering_input_output_aliases=(),
            sim_require_finite=True,
            sim_require_nnan=True,
            nc=nc,
        )
        return tuple(outs)

    devices = jax.devices()[:NCORES]
    mesh = Mesh(np.asarray(devices), ("core",))
    in_specs = (PartitionSpec("core"),) * (n_params + n_outs)
    out_specs = (PartitionSpec("core"),) * n_outs
    donate = tuple(range(n_params, n_params + n_outs))
    sharded = jax.jit(
        shard_map(_body, mesh=mesh, in_specs=in_specs,
                  out_specs=out_specs, check_rep=False),
        donate_argnums=donate, keep_unused=True)

    _STATE.update(dict(
        nc=nc, sharded=sharded, in_names=in_names, out_names=out_names,
        out_avals=out_avals, zero_outs=zero_outs, mesh=mesh))
    return _STATE


def _runner():
    if "sharded" not in _STATE:
        _build_and_jit()
    return _STATE


def make_in_maps(token_inputs, w_router, w1, w2):
    """Per-core input dicts (host-side shard/layout/dtype prep only)."""
    bf = ml_dtypes.bfloat16
    ones_c = np.ones((P, P), np.float32)
    utri_c = np.triu(np.ones((P, P), np.float32))
    iota64 = np.tile(np.arange(E, dtype=np.float32), (P, 8))
    siota = np.tile(np.arange(512, dtype=np.float32), (P, 1))
    consts = np.concatenate([ones_c, utri_c, iota64, siota],
                            axis=1).astype(np.float32)
    wr_p = np.ascontiguousarray(
        w_router.astype(np.float32).reshape(8, P, E)
        .transpose(1, 0, 2).reshape(P, 8 * E))
    in_maps = []
    for g in range(NCORES):
        tg = token_inputs[g].astype(np.float32)          # [T, H]
        tokT = tg.T.reshape(8, P, T)                     # [k, p, t]
        tok_t1 = tokT[:, :, :512].transpose(1, 0, 2).reshape(P, 8 * 512)
        tok_t2 = tokT[:, :, 512:].transpose(1, 0, 2).reshape(P, 8 * 512)
        tok_bp = (tg.astype(bf).reshape(NT, P, H)
                  .transpose(1, 0, 2).reshape(P, NT * H))
        # w1 [H, DFF] -> [p, (mb k c)]: lhsT tiles [p, k(h-tile), c(dff)]
        w1g = w1[g].astype(bf).reshape(8, P, 8, 512)      # [k, p, mb, c]
        w1_p = (w1g.transpose(1, 2, 0, 3)                 # [p, mb, k, c]
                .reshape(P, 8 * DFF))
        # w2 [DFF, H] -> [p, (hq kb k c)]: rhs tiles [p(dff), k, c(h)]
        w2g = w2[g].astype(bf).reshape(4, 8, P, 4, 256)   # [kb, k, p, hq, c]
        w2_p = (w2g.transpose(2, 3, 0, 1, 4)              # [p, hq, kb, k, c]
                .reshape(P, 8 * DFF))
        in_maps.append({
            "tok_t1": np.ascontiguousarray(tok_t1),
            "tok_t2": np.ascontiguousarray(tok_t2),
            "tok_bp": np.ascontiguousarray(tok_bp),
            "wrp": wr_p,
            "w1p": np.ascontiguousarray(w1_p),
            "w2p": np.ascontiguousarray(w2_p),
            "consts": consts,
        })
    return in_maps


def run_in_maps(in_maps):
    st = _runner()
    concat_in = [
        np.concatenate([np.asarray(in_maps[c][name])
                        for c in range(NCORES)], axis=0)
        for name in st["in_names"]
    ]
    concat_zeros = [np.zeros((NCORES * z.shape[0], *z.shape[1:]), z.dtype)
                    for z in st["zero_outs"]]
    out_arrs = st["sharded"](*concat_in, *concat_zeros)
    res = []
    for c in range(NCORES):
        res.append({
            name: np.asarray(out_arrs[i]).reshape(
                NCORES, *st["out_avals"][i].shape)[c]
            for i, name in enumerate(st["out_names"])
        })
    return res


def kernel(token_inputs, w_router, w1, w2, expert_capacity):
    token_inputs = np.asarray(token_inputs)
    w_router = np.asarray(w_router)
    w1 = np.asarray(w1)
    w2 = np.asarray(w2)
    assert int(expert_capacity) == CAP
    assert token_inputs.shape == (G, T, H)
    in_maps = make_in_maps(token_inputs, w_router, w1, w2)
    try:
        res = run_in_maps(in_maps)
    except Exception:
        from concourse import bass_utils
        nc = _STATE.get("nc") or _build_nc()
        res = bass_utils.run_bass_kernel_spmd(
            nc, in_maps, core_ids=list(range(NCORES))).results
    return np.stack([res[g]["out"] for g in range(NCORES)], axis=0)
